# revision 46
# baseline (speedup 1.0000x reference)
"""DecoderAttention Bass/Tile kernel for TRN2, batch-parallel over 8 NeuronCores.

Each core handles one batch element:
  q = enc @ Qs + Qbs ; k = enc @ Ks + Kbs ; v = nrp @ Vs + Vbs   (per head)
  scores = q k^T / sqrt(64), causal mask (-1e5), softmax
  out = (attn @ v) @ O + Ob

Layout/throughput strategy (all matmuls in bf16 at 1 col/cycle, f32 PSUM):
  - all big DRAM inputs pre-cast to bf16 host-side (halves DMA, enables
    bf16 PE transposes and 2x matmul rate vs fp32r; measured end-to-end
    max rel err ~4e-3 vs the 2e-2 gate)
  - enc/nrp transposed on-device (PE transpose, bf16 PSUM) to [d, s];
    nrp first so the v projection can start as early as possible; all
    biases are host-packed into one row-replicated [128, 2080] tensor
    (broadcast/gather DMA patterns cost ~100x more than a dense row load)
  - a burst of dummy matmuls at t=0 holds the PE busy through one full
    HAM activity window, lifting the clock gate to 2.4 GHz early
  - the causal diagonal mask is applied by a gpsimd affine_select zeroing
    the exp output in SBUF, keeping the PE mask matmuls off the stream
  - weights pre-packed host-side to [d, (h dh)]; Vs padded to [d, 16*65]
    with a ones column per head so attn@v also produces softmax row sums
  - scoresT [m, q] per head so exp output feeds attn@v without transposing
  - causal diagonal blocks masked by accumulating I.T @ M0 (bf16) in PSUM
  - exp folds the 1/sqrt(d_head) scale; no max subtraction (scores are O(1),
    masked entries become exactly 0)
  - the HAM clock gate re-throttles whenever PE duty sags inside a ~3.4us
    window, so independent PE work rides a fine-grained filler queue
    threaded through the v-projection and attention loops: q/k projection
    half-groups (4 matmuls) for later pairs, and the softmax-normalization
    broadcast matmuls of earlier pairs (marked late so the PE never blocks
    on their reciprocal chain); units are pair-tagged and force-drained
    before the pair that needs them
  - softmax division deferred: per-pair reciprocal over sums reshaped to
    [16, 128] (partition-parallel reciprocal), broadcast back across
    partitions with 8 one-hot selector matmuls, applied to zt while later
    pairs run; the final pair's normalization overlaps the first output-
    projection accumulations (their k=7 contribution is issued last)
"""

import numpy as np
import ml_dtypes

import concourse.bass as bass
import concourse.mybir as mybir
import concourse.tile as tile
from concourse import bacc
from concourse.bass_utils import run_bass_kernel_spmd
from concourse.masks import make_identity

N_HEADS, D_MODEL, D_HEAD = 16, 1024, 64
BATCH, SEQ = 8, 1024
P = 128
DCH = D_MODEL // P       # 8 contraction chunks
ST = SEQ // P            # 8 seq tiles
PAIRS = N_HEADS // 2     # 8 head pairs
VW = 65                  # v width per head incl. ones column
VTOT = N_HEADS * VW      # 1040
IGNORE = -100000.0
SCALE = 1.0 / np.sqrt(np.float32(D_HEAD))

F32 = mybir.dt.float32
F32R = mybir.dt.float32r
BF16 = mybir.dt.bfloat16
AF = mybir.ActivationFunctionType
NPBF16 = ml_dtypes.bfloat16

_CACHE = {}


def _bank_splits(q0):
    # PSUM-bank-aligned (n0, nw) column splits covering [q0, SEQ)
    if q0 < 512:
        return [(q0, 512 - q0), (512, 512)]
    return [(q0, SEQ - q0)]


def _bcast_row_ap(src, n):
    # DMA access pattern replicating a [n]-element DRAM row to 128 partitions
    return bass.AP(tensor=src.tensor, offset=src.offset, ap=[[0, P], [1, n]])


def _build_program():
    nc = bacc.Bacc("TRN2", target_bir_lowering=False, debug=False, num_devices=8)

    enc = nc.dram_tensor("enc", [SEQ, D_MODEL], BF16, kind="ExternalInput").ap()
    nrp = nc.dram_tensor("nrp", [SEQ, D_MODEL], BF16, kind="ExternalInput").ap()
    qst = nc.dram_tensor("qst", [D_MODEL, D_MODEL], BF16, kind="ExternalInput").ap()
    kst = nc.dram_tensor("kst", [D_MODEL, D_MODEL], BF16, kind="ExternalInput").ap()
    vst = nc.dram_tensor("vst", [D_MODEL, VTOT], BF16, kind="ExternalInput").ap()
    ow = nc.dram_tensor("ow", [D_MODEL, D_MODEL], BF16, kind="ExternalInput").ap()
    # all biases host-packed into one row-replicated [128, 2080] tensor:
    # cols [0:8] qb by pair, [8:16] kb by pair, [16:1056] vb, [1056:2080] ob
    biases = nc.dram_tensor("biases", [P, 16 + VTOT + D_MODEL], F32, kind="ExternalInput").ap()
    out = nc.dram_tensor("out", [SEQ, D_MODEL], BF16, kind="ExternalOutput").ap()
    sums_dram = nc.dram_tensor("sums_scratch", [N_HEADS, SEQ], F32).ap()

    with tile.TileContext(nc) as tc:
        _kernel(tc, out, enc, nrp, qst, kst, vst, ow, biases,
                sums_dram=sums_dram)
    nc.compile()
    return nc


def _kernel(tc, out, enc, nrp, qst, kst, vst, ow, biases, sums_dram=None):
    nc = tc.nc

    smalls = tc.alloc_tile_pool(name="smalls", bufs=1)
    ident_bf = smalls.tile([P, P], BF16, tag="ident_bf", name="ident_bf")
    make_identity(nc, ident_bf)
    # sel[a][j, p] = 1 where j == (p // 64) * 8 + a: broadcasts the [16, 128]
    # reciprocal layout (row = (head, seg), col = q%128) to [128, q-seg a]
    sel = []
    self_f = smalls.tile([N_HEADS, P], F32, tag="self", name="self")
    for a in range(DCH):
        nc.gpsimd.memset(self_f, 0.0)
        nc.gpsimd.affine_select(
            out=self_f.rearrange("j (h c) -> j h c", h=2),
            in_=self_f.rearrange("j (h c) -> j h c", h=2),
            compare_op=mybir.AluOpType.not_equal,
            fill=1.0, base=-a,
            pattern=[[-8, 2], [0, D_HEAD]], channel_multiplier=1,
        )
        s_r = smalls.tile([N_HEADS, P], F32R, tag=f"sel{a}", name=f"sel{a}")
        nc.vector.tensor_copy(s_r, self_f)
        sel.append(s_r)
    bias_sb = smalls.tile([P, 16 + VTOT + D_MODEL], F32, tag="bias_sb", name="bias_sb")
    qb_col = bias_sb[:, 0:PAIRS]
    kb_col = bias_sb[:, PAIRS:2 * PAIRS]
    vb_bc = bias_sb[:, 16:16 + VTOT]
    ob_bc = bias_sb[:, 16 + VTOT:16 + VTOT + D_MODEL]

    # persistent weight tiles (bf16), prefetched on the scalar DMA queue in
    # consumption order (v projection first, output projection last)
    wpool = tc.alloc_tile_pool(name="weights", bufs=1, side="right")
    vw = [wpool.tile([P, VTOT], BF16, tag=f"vw{c}", name=f"vw{c}") for c in range(DCH)]
    qw = [wpool.tile([P, D_MODEL], BF16, tag=f"qw{c}", name=f"qw{c}") for c in range(DCH)]
    kw = [wpool.tile([P, D_MODEL], BF16, tag=f"kw{c}", name=f"kw{c}") for c in range(DCH)]
    owt = [wpool.tile([P, D_MODEL], BF16, tag=f"owt{c}", name=f"owt{c}") for c in range(DCH)]
    for c in range(DCH):
        nc.scalar.dma_start(out=vw[c], in_=vst[c * P:(c + 1) * P, :])
    for c in range(DCH):
        nc.scalar.dma_start(out=qw[c], in_=qst[c * P:(c + 1) * P, :])
    for c in range(DCH):
        nc.scalar.dma_start(out=kw[c], in_=kst[c * P:(c + 1) * P, :])
    for c in range(DCH):
        nc.scalar.dma_start(out=owt[c], in_=ow[c * P:(c + 1) * P, :])

    enc_t_pool = tc.alloc_tile_pool(name="encT", bufs=1, side="right")
    nrp_t_pool = tc.alloc_tile_pool(name="nrpT", bufs=1, side="right")
    encT = [enc_t_pool.tile([P, SEQ], BF16, tag=f"encT{c}", name=f"encT{c}") for c in range(DCH)]
    nrpT = [nrp_t_pool.tile([P, SEQ], BF16, tag=f"nrpT{c}", name=f"nrpT{c}") for c in range(DCH)]

    # ---- phase 1: dummy-matmul burst to lift the HAM clock gate, then
    #      transpose nrp into [d, s] (bf16). enc is transposed later,
    #      interleaved into the v projection so its DMA latency hides
    #      behind matmul work. Chunk-grouped 3/3/2 so the transpose PSUM
    #      tiles fit alongside the v-projection accumulator. ----
    with tc.tile_pool(name="warmps", bufs=1, space="PSUM") as wps:
        warm = wps.tile([P, P], F32, tag="warm", name="warm")
        for _ in range(32):
            nc.tensor.matmul(warm, ident_bf, ident_bf, start=True, stop=True,
                             skip_group_check=True)
    CG = ((0, 1, 2), (3, 4, 5), (6, 7))
    trin = tc.alloc_tile_pool(name="trin", bufs=8, side="right")
    pproj = tc.alloc_tile_pool(name="pproj", bufs=2, space="PSUM")
    trps = tc.alloc_tile_pool(name="trps", bufs=1, space="PSUM")

    def transpose_block(src, dst, tq):
        # two seq-tiles of src -> dst[c][:, tq*P:(tq+2)*P]
        s_ins = []
        for t in range(tq, tq + 2):
            s_in = trin.tile([P, D_MODEL], BF16, tag="s_in", name="s_in")
            nc.sync.dma_start(out=s_in, in_=src[t * P:(t + 1) * P, :])
            s_ins.append(s_in)
        for cg in CG:
            ptiles = {c: trps.tile([P, 2 * P], BF16, tag=f"tr{ci}", name=f"tr{ci}")
                      for ci, c in enumerate(cg)}
            for ti, s_in in enumerate(s_ins):
                for c in cg:
                    nc.tensor.transpose(
                        ptiles[c][:, ti * P:(ti + 1) * P],
                        s_in[:, c * P:(c + 1) * P],
                        ident_bf,
                    )
            for c in cg:
                nc.any.tensor_copy(dst[c][:, tq * P:(tq + 2) * P], ptiles[c])

    for tq in range(0, ST, 2):
        transpose_block(nrp, nrpT, tq)
    nc.sync.dma_start(out=bias_sb, in_=biases)

    qt_pool = tc.alloc_tile_pool(name="qt", bufs=1)
    kt_pool = tc.alloc_tile_pool(name="kt", bufs=1)
    qt = [qt_pool.tile([P, SEQ], BF16, tag=f"qt{g}", name=f"qt{g}") for g in range(PAIRS)]
    kt = [kt_pool.tile([P, SEQ], BF16, tag=f"kt{g}", name=f"kt{g}") for g in range(PAIRS)]
    va_pool = tc.alloc_tile_pool(name="va", bufs=1)
    va = [va_pool.tile([P, VTOT], BF16, tag=f"va{t}", name=f"va{t}") for t in range(ST)]
    zt_pool = tc.alloc_tile_pool(name="zt", bufs=1)
    zt = [zt_pool.tile([P, SEQ], BF16, tag=f"zt{k}", name=f"zt{k}") for k in range(DCH)]

    rpool = tc.alloc_tile_pool(name="rnorm", bufs=1)

    # ---- PE filler queue: (pair, early_ok, emit). Early slots sit right
    # behind fresh attention matmuls, so only latency-free work goes there.
    # Units are pair-tagged so everything pair g needs is force-drained
    # before its attention begins. ----
    filler = []
    proj_state = {}

    def proj_half_mms(g, w, bcol, dst, n0, chalf):
        key = (g, id(w), n0)
        if chalf == 0:
            proj_state[key] = pproj.tile([P, 512], F32, tag="pp", name="pp")
        pp = proj_state[key]
        for c in range(4 * chalf, 4 * chalf + 4):
            nc.tensor.matmul(
                pp,
                w[c][:, g * P:(g + 1) * P],
                encT[c][:, n0:n0 + 512],
                start=(c == 0), stop=(c == DCH - 1),
                skip_group_check=True,
            )
        if chalf == 1:
            del proj_state[key]
            nc.vector.tensor_scalar_add(
                out=dst[g][:, n0:n0 + 512], in0=pp, scalar1=bcol[:, g:g + 1],
            )

    def push_proj_pair(g):
        for w, bcol, dst in ((qw, qb_col, qt), (kw, kb_col, kt)):
            for n0 in (0, 512):
                for chalf in (0, 1):
                    filler.append((g, True,
                                   lambda g=g, w=w, bcol=bcol, dst=dst, n0=n0, chalf=chalf:
                                   proj_half_mms(g, w, bcol, dst, n0, chalf)))

    def push_norm_pair(pg):
        # reciprocal of softmax sums for pair pg: gather the two sum rows as
        # [16, 128] so the FD-bound reciprocal runs across partitions, then
        # broadcast to [128, SEQ] via the one-hot selector matmuls.
        s2 = rpool.tile([N_HEADS, P], F32, tag="s2", name="s2", bufs=2)
        nc.sync.dma_start(
            out=s2,
            in_=sums_dram[2 * pg:2 * pg + 2, :].rearrange("h (a c) -> (h a) c", c=P),
        )
        r2 = rpool.tile([N_HEADS, P], F32R, tag="r2", name="r2", bufs=2)
        with nc.allow_low_precision(reason="softmax denominators are O(1); fp32r rounding is fine"):
            nc.vector.reciprocal(out=r2, in_=s2)

        def apply(half, r2=r2, pg=pg):
            pb = pproj.tile([P, 512], F32, tag="pp", name="ppb")
            for a in range(4 * half, 4 * half + 4):
                nc.tensor.matmul(
                    pb[:, (a % 4) * P:(a % 4 + 1) * P], sel[a], r2,
                    start=True, stop=True, skip_group_check=True,
                )
            nc.vector.tensor_mul(
                zt[pg][:, half * 512:half * 512 + 512],
                zt[pg][:, half * 512:half * 512 + 512],
                pb,
            )

        for half in (0, 1):
            filler.append((None, False, lambda half=half: apply(half)))

    def pop_filler(allow_late):
        for idx, (pg, early_ok, emit) in enumerate(filler):
            if early_ok or allow_late:
                filler.pop(idx)
                emit()
                return

    def drain_pair(g):
        # everything pair g depends on must be emitted before its attention
        mine = [u for u in filler if u[0] == g]
        filler[:] = [u for u in filler if u[0] != g]
        for u in mine:
            u[2]()

    # ---- phase 2: v projection -> va [m, 16*65] with ones columns.
    # enc transpose blocks and the q/k projections for pairs 0 and 1 are
    # interleaved so the PE never idles on the enc DMA stream. The n0=0
    # projection halves only need enc seq-tiles 0-3, which are transposed
    # by t=3; n0=512 halves are force-drained at attention start. ----
    for g01 in (0, 1):
        for w, bcol, dst in ((qw, qb_col, qt), (kw, kb_col, kt)):
            for chalf in (0, 1):
                filler.append((g01, True,
                               lambda g=g01, w=w, bcol=bcol, dst=dst, chalf=chalf:
                               proj_half_mms(g, w, bcol, dst, 0, chalf)))
    for g01 in (0, 1):
        for w, bcol, dst in ((qw, qb_col, qt), (kw, kb_col, kt)):
            for chalf in (0, 1):
                filler.append((g01, True,
                               lambda g=g01, w=w, bcol=bcol, dst=dst, chalf=chalf:
                               proj_half_mms(g, w, bcol, dst, 512, chalf)))
    with tc.tile_pool(name="pv", bufs=1, space="PSUM") as pv:
        for t in range(ST):
            pt = pv.tile([P, VTOT], F32, tag="pv", name="pvt")

            def vmm(c, n0, nw, t=t, pt=pt):
                nc.tensor.matmul(
                    pt[:, n0:n0 + nw],
                    nrpT[c][:, t * P:(t + 1) * P],
                    vw[c][:, n0:n0 + nw],
                    start=(c == 0), stop=(c == DCH - 1),
                    skip_group_check=True,
                )

            # the 16-col ones-tail matmul is delayed one chunk so its fill
            # hides between large matmuls on other PSUM regions (emitting it
            # in place serializes the chunk at ~916ns instead of ~460ns)
            tail = None
            for c in range(DCH):
                vmm(c, 0, 512)
                vmm(c, 512, 512)
                if tail is not None:
                    vmm(tail, 1024, 16)
                tail = c
            vmm(tail, 1024, 16)
            if t < 4:
                transpose_block(enc, encT, 2 * t)
            else:
                pop_filler(allow_late=True)
                pop_filler(allow_late=True)
            # vb_bc has the per-(h,dh) bias, with 1.0 in each ones-column slot;
            # matmul wrote 0 there (vst ones-columns are zero), so add gives 1.0
            nc.vector.tensor_add(va[t], pt, vb_bc)

    trps.release()
    trin.release()
    nrp_t_pool.release()

    # ---- phase 3: attention; later pairs' q/k projections and earlier
    #      pairs' softmax normalization ride the PE filler queue ----
    with tc.tile_pool(name="attn", bufs=3) as apool, \
         tc.tile_pool(name="ps_s", bufs=2, space="PSUM") as spool, \
         tc.tile_pool(name="ps_z", bufs=1, space="PSUM") as zpool:
        for h in range(N_HEADS):
            g, off = h // 2, (h % 2) * D_HEAD
            if h % 2 == 0:
                drain_pair(g)
                if g >= 1:
                    push_norm_pair(g - 1)
                if g + 2 < PAIRS:
                    push_proj_pair(g + 2)
            pz = zpool.tile([VW, SEQ], F32, tag="pz", name="pz")

            def av_mms(i, ae):
                q0 = i * P
                for n0, nw in _bank_splits(q0):
                    nc.tensor.matmul(
                        pz[:, n0:n0 + nw],
                        va[i][:, h * VW:(h + 1) * VW],
                        ae[:, n0:n0 + nw],
                        start=(i == 0), stop=(i == ST - 1),
                        skip_group_check=True,
                    )

            pend = None
            for i in range(ST):
                q0 = i * P
                ps = spool.tile([P, SEQ], F32, tag="ps", name="ps")
                ae = apool.tile([P, SEQ], BF16, tag="ae", name="ae")
                for n0, nw in _bank_splits(q0):
                    nc.tensor.matmul(
                        ps[:, n0:n0 + nw],
                        kt[g][off:off + D_HEAD, q0:q0 + P],
                        qt[g][off:off + D_HEAD, n0:n0 + nw],
                        start=True, stop=True,
                        skip_group_check=True,
                    )
                if i % 2 == 0:
                    pop_filler(allow_late=(i >= 4))
                nc.scalar.activation(
                    out=ae[:, q0:SEQ], in_=ps[:, q0:SEQ],
                    func=AF.Exp, scale=float(SCALE),
                )
                # causal diag mask: zero ae[m, q] where m > q (gpsimd, off
                # the PE critical path; exp of unmasked scores is harmless)
                nc.gpsimd.affine_select(
                    out=ae[:, q0:q0 + P], in_=ae[:, q0:q0 + P],
                    compare_op=mybir.AluOpType.is_ge,
                    fill=0.0, base=0,
                    pattern=[[1, P]], channel_multiplier=-1,
                )
                # attn@v delayed one chunk so exp latency hides behind PE work
                if pend is not None:
                    av_mms(*pend)
                pend = (i, ae)
            av_mms(*pend)
            # denominator row first (feeds the normalization chain), then
            # stash unnormalized zT; both free the PSUM slot
            srow = rpool.tile([1, SEQ], F32, tag="srow", name="srow", bufs=2)
            nc.vector.tensor_copy(srow, pz[D_HEAD:VW, :])
            nc.sync.dma_start(out=sums_dram[h:h + 1, :], in_=srow)
            nc.vector.tensor_copy(zt[g][off:off + D_HEAD, :], pz[0:D_HEAD, :])

        push_norm_pair(PAIRS - 1)

    # ---- phase 4: output projection out[s, d] = zt.T @ O + ob.
    # The first two seq-tiles' k=0..6 accumulations overlap the final
    # pair's normalization chain; their k=7 matmuls are issued after it.
    with tc.tile_pool(name="outsb", bufs=3) as outsb, \
         tc.tile_pool(name="po", bufs=2, space="PSUM") as po:
        def out_mms(pt, t, k):
            for n0 in range(0, D_MODEL, 512):
                nc.tensor.matmul(
                    pt[:, n0:n0 + 512],
                    zt[k][:, t * P:(t + 1) * P],
                    owt[k][:, n0:n0 + 512],
                    start=(k == 0), stop=(k == DCH - 1),
                    skip_group_check=True,
                )

        def out_drain(pt, t):
            # bf16 output (upcast host-side) halves the writeback traffic,
            # striped over three DMA queues so the tail drains fast
            ot = outsb.tile([P, D_MODEL], BF16, tag="ot", name="ot")
            nc.vector.tensor_add(ot, pt, ob_bc)
            eng = (nc.sync, nc.scalar, nc.gpsimd)[t % 3]
            eng.dma_start(out=out[t * P:(t + 1) * P, :], in_=ot)

        head_pts = []
        for t in (0, 1):
            pt = po.tile([P, D_MODEL], F32, tag="po", name="pot")
            head_pts.append(pt)
            for k in range(DCH - 1):
                out_mms(pt, t, k)
        while filler:
            pop_filler(allow_late=True)
        for t in (0, 1):
            out_mms(head_pts[t], t, DCH - 1)
            out_drain(head_pts[t], t)
        for t in range(2, ST):
            pt = po.tile([P, D_MODEL], F32, tag="po", name="pot")
            for k in range(DCH):
                out_mms(pt, t, k)
            out_drain(pt, t)

    pproj.release()
    for pool in (rpool, zt_pool, va_pool, kt_pool, qt_pool, enc_t_pool, wpool, smalls):
        pool.release()


def _get_program():
    if "nc" not in _CACHE:
        _CACHE["nc"] = _build_program()
    return _CACHE["nc"]


def _pack_weights(Qs, Qbs, Ks, Kbs, Vs, Vbs, O, Ob):
    f = np.float32
    qst = np.ascontiguousarray(np.transpose(np.asarray(Qs, f), (1, 0, 2)).reshape(D_MODEL, D_MODEL)).astype(NPBF16)
    kst = np.ascontiguousarray(np.transpose(np.asarray(Ks, f), (1, 0, 2)).reshape(D_MODEL, D_MODEL)).astype(NPBF16)
    vst = np.zeros((D_MODEL, VTOT), f)
    vb = np.zeros((VTOT,), f)
    Vs = np.asarray(Vs, f)
    Vbs = np.asarray(Vbs, f)
    for h in range(N_HEADS):
        vst[:, h * VW:h * VW + D_HEAD] = Vs[h]
        vb[h * VW:h * VW + D_HEAD] = Vbs[h]
        vb[h * VW + D_HEAD] = 1.0
    vst = vst.astype(NPBF16)
    ow = np.ascontiguousarray(np.asarray(O, f).reshape(D_MODEL, D_MODEL)).astype(NPBF16)
    qbf = np.asarray(Qbs, f).reshape(D_MODEL)
    kbf = np.asarray(Kbs, f).reshape(D_MODEL)
    obf = np.asarray(Ob, f).reshape(D_MODEL)
    # row-replicated bias pack: [0:8] qb by (pair, partition), [8:16] kb,
    # [16:1056] vb broadcast, [1056:2080] ob broadcast
    biases = np.empty((P, 16 + VTOT + D_MODEL), f)
    biases[:, 0:PAIRS] = qbf.reshape(PAIRS, P).T
    biases[:, PAIRS:2 * PAIRS] = kbf.reshape(PAIRS, P).T
    biases[:, 16:16 + VTOT] = vb[None, :]
    biases[:, 16 + VTOT:] = obf[None, :]
    return qst, kst, vst, ow, np.ascontiguousarray(biases)


def kernel(normalized_resid_pre, encoder_output, Qs, Qbs, Ks, Kbs, Vs, Vbs, O, Ob,
           _trace=False, _trace_kwargs=None):
    nc = _get_program()
    qst, kst, vst, ow, biases = _pack_weights(Qs, Qbs, Ks, Kbs, Vs, Vbs, O, Ob)
    enc = np.asarray(encoder_output, np.float32).astype(NPBF16)
    nrp = np.asarray(normalized_resid_pre, np.float32).astype(NPBF16)
    in_maps = []
    for b in range(BATCH):
        in_maps.append({
            "enc": np.ascontiguousarray(enc[b]),
            "nrp": np.ascontiguousarray(nrp[b]),
            "qst": qst, "kst": kst, "vst": vst, "ow": ow,
            "biases": biases,
        })
    res = run_bass_kernel_spmd(
        nc, in_maps, list(range(BATCH)),
        trace=_trace, **(_trace_kwargs or {}),
    )
    out = np.stack([np.asarray(res.results[b]["out"]).astype(np.float32) for b in range(BATCH)], axis=0)
    if _trace:
        _CACHE["last_results"] = res
    return out
